# revision 2
# baseline (speedup 1.0000x reference)
"""Trainium2 Bass kernel for ClauseBodyInferModule (gnn_message_passing).

out[c, b, g] = sum_s prod_l x[b, I[c, g, s, l]]
  x: [B=32, G=8192] f32, I: [C=16, G=8192, S=8, L=3] int, out: [C, B, G] f32.

Sharding: clauses split across the 8 NeuronCores (2 clauses/core).

Per-core strategy — GPSIMD ap_gather main path + SWDGE dma_gather sideline:
  * x is packed on host into a [128, G] int32 table: partition p holds the
    bf16 pair (x[2*(p%16)], x[2*(p%16)+1]) per ground atom, so each group of
    16 partitions (one GPSIMD core) covers the full batch B=32.
  * The (c,g,s,l) slot list is split 8 ways across the GPSIMD cores; each
    core's slice is laid out wrapped over its 16 partitions exactly as
    InstAPGather consumes it (host-side pure index shuffling).
  * nc.gpsimd.ap_gather pulls, per list slot, the packed batch-pair int32
    from each of the 16 channels: one Pool pass covers all 32 batch
    elements.  5 chunks, double buffered.
  * While the 4MB table uploads (Pool would be idle), Pool desc-gens four
    SWDGE dma_gather calls that fetch the last 128 (c,g) of every core's
    range straight from a row-major DRAM copy of x (bf16, 256B row stride);
    their DMA transfers drain right after the upload.
  * DVE forms the L-conjunction with two packed-bf16 multiplies and the
    S-sum with a bf16 pairwise add tree (f32 final), writing layouts that
    give the output DMAs contiguous runs.
"""

import os

os.environ.setdefault("TILE_SCHEDULER", "asap")

import numpy as np

import concourse.ap_utils as ap_utils
from concourse import bacc, mybir, tile
from concourse.bass import MemorySpace
from concourse.bass_utils import run_bass_kernel_spmd

C, G, S, L, B = 16, 8192, 8, 3, 32
NCORES = 8
CC = C // NCORES          # clauses per NeuronCore
P = 128
NGC = 8                   # gpsimd cores (16-partition groups) per NeuronCore
CGC = CC * G // NGC       # (c,g) pairs per gpsimd core (2048)
APC = 1888                # (c,g) pairs per core on the ap_gather path
HYB = CGC - APC           # (c,g) pairs per core on the dma_gather path (160)
# ap_gather chunk sizes in (c,g) pairs; multiples of 8 so every chunk's index
# slice is a whole number of 32 int16 (the ucode reads indices as 16-lane
# uint32 vectors from a 4-byte-aligned base).  Every chunk keeps
# num_idxs >= 8192 (the instruction's cost floor is the table's free size);
# sizes shrink toward the end so the DVE drain after the last gather is short.
MCH = [400, 392, 384, 368, 344]
MOFF = [0, 400, 792, 1176, 1544]
KCH = [24 * m for m in MCH]
WOFF = [0, 600, 1188, 1764, 2316, 2832]     # idx word offsets (cumsum K/16)
NW = WOFF[-1]
MMAX = max(MCH)
KMAX = max(KCH)
# dma_gather sideline: one queue call per clause; partition p covers 5 g's
# of q-range p//32 (g = 2048*(p//32) + APC_g + 5*(p%32) + mh), 24 slots per g.
NQ = 4
NHC = 2                    # hybrid calls (queues used)
HM = 5                     # g's per partition per call
HJJ = HM * S * L           # gathered rows per partition per call (120)
NIH = HJJ * P              # list slots per call (15360)
HW = NIH // 16             # idx words per queue-group partition (960)

_cache: dict = {}


def _dma_gather_rows(gp, out_ap, in_ap, idxs_ap, num_idxs, elem_size, elem_step, queue_num):
    """nc.gpsimd.dma_gather minus the elem_size%256 assert (non-transpose,
    DRAM source).  HW only requires the row *stride* to be a 256B multiple;
    elem_size can be smaller (the tail of each row is simply not read)."""
    assert idxs_ap.dtype == mybir.dt.int16
    assert in_ap.space == MemorySpace.DRAM
    assert in_ap.dtype == out_ap.dtype
    assert ap_utils.ap_is_contiguous(out_ap.ap[1:])
    assert ap_utils.ap_is_contiguous(idxs_ap.ap[1:])
    assert in_ap.ap[-1][1] == elem_size and out_ap.ap[-1][1] == elem_size
    assert out_ap.ap[0][1] * out_ap.ap[1][1] == -(-num_idxs // 128) * 128
    assert in_ap.ap[0][0] == elem_step
    stride_bytes = elem_step * mybir.dt.size(in_ap.dtype)
    stride_bytes_256, rem = divmod(stride_bytes, 256)
    assert rem == 0 and stride_bytes_256 < 256
    _in_ap = gp.lower_ap_dma(in_ap, for_custom_bir_dma=True)
    _idxs_ap = gp.lower_ap(idxs_ap)
    _out_ap = gp.lower_ap(out_ap)
    return gp.add_instruction(
        mybir.InstDMAGatherAnt(
            name=gp.bass.get_next_instruction_name(),
            ins=[*_in_ap, _idxs_ap, gp.lower_val_access(gp.to_reg(num_idxs))],
            outs=[_out_ap],
            transpose=False,
            num_idxs=num_idxs,
            elem_size=elem_size,
            stride_bytes_256=stride_bytes_256,
            gen_mode=0,
            single_packet=False,
            queue_num=queue_num,
            sbuf_tokens_per_rank=0,
            sbuf_free_dim_per_rank=0,
            sbuf_free_dim_pad_per_rank=0,
            sbuf_byte_offset=0,
        )
    )


def _build(idx_is_64: bool):
    del idx_is_64  # host converts indices to int16 either way
    nc = bacc.Bacc(
        "TRN2",
        target_bir_lowering=False,
        debug=False,
        num_devices=NCORES,
        num_swdge_queues=NQ,
    )
    f32 = mybir.dt.float32
    i32 = mybir.dt.int32
    i16 = mybir.dt.int16
    bf16 = mybir.dt.bfloat16
    mult = mybir.AluOpType.mult
    add = mybir.AluOpType.add

    xt = nc.dram_tensor("xt", [P, G], i32, kind="ExternalInput").ap()
    xt2 = nc.dram_tensor("xt2", [G, P], i16, kind="ExternalInput").ap()
    idx = nc.dram_tensor("idx", [P, NW], i16, kind="ExternalInput").ap()
    hidx = nc.dram_tensor("hidx", [P, HW], i16, kind="ExternalInput").ap()
    out = nc.dram_tensor("out", [CC, B, G], f32, kind="ExternalOutput").ap()
    # ap path: partition (c*4 + q)*16 + t holds batches (2t, 2t+1) of clause
    # c, g = 2048q + MOFF[chunk] + m.
    ovv = out.rearrange("c (t e) (q gg) -> c q t e gg", e=2, q=4)

    with tile.TileContext(nc) as tc:
        with tc.tile_pool(name="tabp", bufs=1) as tabp, tc.tile_pool(
            name="idxp", bufs=1
        ) as idxp, tc.tile_pool(name="gathp", bufs=2) as gathp, tc.tile_pool(
            name="bodyp", bufs=2
        ) as bodyp, tc.tile_pool(name="s4p", bufs=2) as s4p, tc.tile_pool(
            name="s2p", bufs=3
        ) as s2p, tc.tile_pool(name="ofp", bufs=5) as ofp, tc.tile_pool(
            name="hybp", bufs=2
        ) as hybp, tc.tile_pool(name="hstp", bufs=2) as hstp:
            table = tabp.tile([P, G], i32, name="table")
            idxs = idxp.tile([P, NW], i16, name="idxs", tag="idxs")
            hidxs = idxp.tile([P, HW], i16, name="hidxs", tag="hidxs")
            # upload order: sideline lists, first two ap chunks' lists, the
            # 4MB table (the ap-path gate), then the remaining lists
            nc.sync.dma_start(out=hidxs[:], in_=hidx[:])
            nc.sync.dma_start(out=idxs[:, WOFF[0]:WOFF[1]], in_=idx[:, WOFF[0]:WOFF[1]])
            nc.sync.dma_start(out=table[:], in_=xt[:])
            for t in range(1, len(MCH)):
                nc.sync.dma_start(
                    out=idxs[:, WOFF[t]:WOFF[t + 1]], in_=idx[:, WOFF[t]:WOFF[t + 1]]
                )
            # ---- dma_gather sideline: desc-gen fills the upload window
            for qn in range(NHC):
                c = qn
                gh = hybp.tile([P, HJJ * B], i16, name=f"gh{qn}", tag="gh")
                _dma_gather_rows(
                    nc.gpsimd,
                    gh[:].rearrange("p (j b) -> p j b", b=B),
                    xt2[:, :B],
                    hidxs[:],
                    num_idxs=NIH,
                    elem_size=B,
                    elem_step=P,
                    queue_num=qn,
                )
                # gh[p, (mh s l b)] bf16: conjunction + S-sum tree
                gvh = gh[:].bitcast(bf16).rearrange(
                    "p (mh s l b) -> p mh s l b", mh=HM, s=S, l=L, b=B
                )
                bh = hybp.tile([P, HM * S * B], bf16, name=f"bh{qn}", tag="bh")
                bvh = bh[:].rearrange("p (mh s b) -> p mh s b", mh=HM, s=S, b=B)
                nc.vector.tensor_tensor(
                    out=bvh, in0=gvh[:, :, :, 0, :], in1=gvh[:, :, :, 1, :], op=mult
                )
                nc.vector.tensor_tensor(
                    out=bvh, in0=bvh, in1=gvh[:, :, :, 2, :], op=mult
                )
                s4h = hybp.tile([P, HM * 4 * B], bf16, name=f"s4h{qn}", tag="s4h")
                bph = bh[:].rearrange("p (mh u v b) -> p mh u v b", mh=HM, u=4, v=2, b=B)
                s4vh = s4h[:].rearrange("p (mh u b) -> p mh u b", mh=HM, u=4, b=B)
                nc.vector.tensor_tensor(
                    out=s4vh, in0=bph[:, :, :, 0, :], in1=bph[:, :, :, 1, :], op=add
                )
                s2h = hybp.tile([P, HM * 2 * B], bf16, name=f"s2h{qn}", tag="s2h")
                s4wh = s4h[:].rearrange("p (mh w v b) -> p mh w v b", mh=HM, w=2, v=2, b=B)
                s2vh = s2h[:].rearrange("p (mh w b) -> p mh w b", mh=HM, w=2, b=B)
                nc.vector.tensor_tensor(
                    out=s2vh, in0=s4wh[:, :, :, 0, :], in1=s4wh[:, :, :, 1, :], op=add
                )
                # stage layout [b][mh] so the out DMA moves 8B m-runs
                sth = hstp.tile([P, B * HM], f32, name=f"sth{qn}", tag="sth")
                stv = sth[:].rearrange("p (b mh) -> p mh b", b=B, mh=HM)
                s2wh = s2h[:].rearrange("p (mh v b) -> p mh v b", mh=HM, v=2, b=B)
                nc.vector.tensor_tensor(
                    out=stv, in0=s2wh[:, :, 0, :], in1=s2wh[:, :, 1, :], op=add
                )
                # out[c, b, 2048*q + 1888 + 5*ph + m], partitions grouped by q
                for d in range(4):
                    dst = out[c].rearrange("b (q j) -> q j b", q=4)[
                        d, 2048 - HM * 32:2048
                    ].rearrange("(ph m) b -> ph b m", m=HM)
                    nc.sync.dma_start(
                        out=dst,
                        in_=sth[32 * d:32 * (d + 1)].rearrange(
                            "p (b mh) -> p b mh", b=B, mh=HM
                        ),
                    )
            # ---- main path: ap_gather chunks
            for t in range(len(MCH)):
                m, K = MCH[t], KCH[t]
                gath = gathp.tile([P, KMAX], i32, name=f"gath{t}", tag="gath")
                nc.gpsimd.ap_gather(
                    gath[:, :K],
                    table[:],
                    idxs[:, WOFF[t]:WOFF[t + 1]],
                    channels=P,
                    num_elems=G,
                    d=1,
                    num_idxs=K,
                )
                # gathered bf16 view: elem 2*(m*24 + s*3 + l) + e
                gv = gath[:, :K].bitcast(bf16).rearrange(
                    "p (m s l e) -> p m s l e", m=m, s=S, l=L, e=2
                )
                body = bodyp.tile([P, MMAX * S * 2], bf16, name=f"body{t}", tag="body")
                bv = body[:, : m * S * 2].rearrange("p (m s e) -> p m s e", s=S, e=2)
                nc.vector.tensor_tensor(
                    out=bv, in0=gv[:, :, :, 0, :], in1=gv[:, :, :, 1, :], op=mult
                )
                nc.vector.tensor_tensor(
                    out=bv, in0=bv, in1=gv[:, :, :, 2, :], op=mult
                )
                # S-sum as a bf16 pairwise tree (f32 final)
                s4 = s4p.tile([P, MMAX * 4 * 2], bf16, name=f"s4_{t}", tag="s4")
                bp = body[:, : m * S * 2].rearrange(
                    "p (m u v e) -> p m u v e", u=4, v=2, e=2
                )
                s4v = s4[:, : m * 4 * 2].rearrange("p (m u e) -> p m u e", u=4, e=2)
                nc.vector.tensor_tensor(
                    out=s4v, in0=bp[:, :, :, 0, :], in1=bp[:, :, :, 1, :], op=add
                )
                s2 = s2p.tile([P, MMAX * 2 * 2], bf16, name=f"s2_{t}", tag="s2")
                s4w = s4[:, : m * 4 * 2].rearrange(
                    "p (m w v e) -> p m w v e", w=2, v=2, e=2
                )
                s2v = s2[:, : m * 2 * 2].rearrange("p (m w e) -> p m w e", w=2, e=2)
                nc.vector.tensor_tensor(
                    out=s2v, in0=s4w[:, :, :, 0, :], in1=s4w[:, :, :, 1, :], op=add
                )
                of = ofp.tile([P, 2 * MMAX], f32, name=f"of{t}", tag="of")
                ofv = of[:, : 2 * m].rearrange("p (e m) -> p m e", e=2)
                s2w = s2[:, : m * 2 * 2].rearrange("p (m v e) -> p m v e", v=2, e=2)
                nc.vector.tensor_tensor(
                    out=ofv, in0=s2w[:, :, 0, :], in1=s2w[:, :, 1, :], op=add
                )
                for c in range(CC):
                    nc.sync.dma_start(
                        out=ovv[c, :, :, :, MOFF[t]:MOFF[t] + m],
                        in_=of[64 * c:64 * (c + 1), : 2 * m].rearrange(
                            "p (e m) -> p e m", e=2
                        ),
                    )
    nc.compile()
    return nc


def _get(idx_is_64: bool):
    if idx_is_64 not in _cache:
        _cache[idx_is_64] = _build(idx_is_64)
    return _cache[idx_is_64]


def _bf16_bits(a):
    u = np.ascontiguousarray(np.asarray(a, np.float32)).view(np.uint32)
    return ((u + 0x7FFF + ((u >> 16) & 1)) >> 16).astype(np.uint16)


def _make_in_maps(x, I):
    x = np.asarray(x)
    I = np.asarray(I)
    xb = _bf16_bits(x)                       # [B, G] bf16 bit patterns
    tsel = np.arange(P) % 16
    lo = xb[2 * tsel].astype(np.uint32)      # [128, G]
    hi = xb[2 * tsel + 1].astype(np.uint32)
    xt = np.ascontiguousarray(lo | (hi << 16)).view(np.int32)
    xt2 = np.zeros((G, P), np.int16)
    xt2[:, :B] = np.ascontiguousarray(xb.T).view(np.int16)
    idx16 = I.astype(np.int16)               # values < 8192 fit
    maps = []
    qg = G // 4                              # 2048, g's per gpsimd-core range
    for i in range(NCORES):
        sub = idx16[i * CC:(i + 1) * CC]     # [CC, G, S, L]
        # ap_gather lists: first APC (c,g) of every core's 2048-range
        lay = np.empty((P, NW), np.int16)
        for k in range(NGC):
            c, q = divmod(k, 4)
            rows = sub[c, qg * q:qg * q + APC].reshape(-1)
            for t in range(len(MCH)):
                seg = rows[24 * MOFF[t]: 24 * (MOFF[t] + MCH[t])]
                # list position n -> [partition n%16, word n//16]
                lay[16 * k:16 * (k + 1), WOFF[t]:WOFF[t + 1]] = seg.reshape(-1, 16).T
        # dma_gather lists: the last HYB/4 g's of every q-range.  Call qn
        # covers clause qn: slot n = jj*128 + p, jj = (mh*8+s)*3 + l,
        # g = qg*(p//32) + 1888 + 5*(p%32) + mh
        hlay = np.zeros((P, HW), np.int16)
        gbase = qg - HM * 32                 # 1888 within each q-range
        pp = np.arange(P)
        for qn in range(NHC):
            c = qn
            gidx = qg * (pp // 32) + gbase + HM * (pp % 32)         # [P]
            # V[jj, p]: jj = (mh*8 + s)*3 + l
            Vg = sub[c][(gidx[None, :] + np.arange(HM)[:, None])]   # [HM, P, S, L]
            V = Vg.transpose(0, 2, 3, 1).reshape(HJJ, P)            # [(mh s l), P]
            w = V.reshape(HJJ, 8, 16).transpose(2, 0, 1).reshape(16, HW)
            hlay[32 * qn:32 * qn + 16] = w
            hlay[32 * qn + 16:32 * qn + 32] = w
        maps.append({
            "xt": xt, "xt2": xt2,
            "idx": np.ascontiguousarray(lay),
            "hidx": np.ascontiguousarray(hlay),
        })
    return maps


def kernel(x, I):
    x = np.asarray(x)
    I = np.asarray(I)
    nc = _get(I.dtype == np.int64)
    res = run_bass_kernel_spmd(
        nc, _make_in_maps(x, I), core_ids=list(range(NCORES))
    )
    return np.concatenate(
        [res.results[i]["out"] for i in range(NCORES)], axis=0
    )
